# revision 37
# baseline (speedup 1.0000x reference)
"""Trainium2 Bass kernel for nn_BERT_61873298866553.

6-layer pre-norm BERT encoder (B=2, S=1024, D=1024, H=16, DF=4096) with a
3-layer input MLP and a 2-layer output head.

Distribution: 8-way sequence sharding (core i owns batch i//4, tokens
(i%4)*256..+256).  Everything is token-local except attention K/V, which is
all-gathered per layer inside the two 4-core batch groups
(replica_groups=[[0..3],[4..7]]) as ONE merged K+V collective.

On-device layout: activations are feature-major (features on SBUF
partitions, tokens on the free axis), so every linear is
out^T[of,t] = sum_ci W[ci,of]^T @ x^T[ci,t] with W chunks stationary.
GEMMs run in bf16 with fp32 PSUM accumulation; the residual stream,
LayerNorm and softmax statistics stay fp32.  LN reduces over the feature
(partition) axis with fp32 ones-matmuls; per-token stats are broadcast via
gpsimd partition_broadcast (which requires base-partition-0 inputs on HW).

Attention: heads processed in pairs (h, h+1) sharing one [128,512] PSUM
score bank -> one exp per pair.  V is stored in a persistent [128,KC,H,65]
SBUF tile whose slot column 0 is a constant 1.0 (memset once), so the
softmax denominator rides the PV matmul as PSUM row 0 -- no separate
denominator matmuls.  Per pair: one reciprocal_approx_fast on [1,512],
one partition_broadcast, one tensor_mul.  Wo is consumed as 16 chunks of
65 rows whose row 0 is zero (kills the garbage dn*recip row).
"""

import sys

if "/opt/trn_rl_repo" not in sys.path:
    sys.path.insert(0, "/opt/trn_rl_repo")

import numpy as np
import ml_dtypes

import concourse.bass as bass
import concourse.tile as tile
import concourse.mybir as mybir
from concourse import bacc
from concourse import bass_utils

F32 = mybir.dt.float32
BF16 = mybir.dt.bfloat16
FP8 = mybir.dt.float8e4
AF = mybir.ActivationFunctionType
ALU = mybir.AluOpType

# Model dims (fixed by the problem).
B, S, IN = 2, 1024, 64
D, H, NL, DF = 1024, 16, 6, 4096
DK = D // H          # 64
DR = D // 4          # 256
EPS = 1e-5
SCALE = 1.0 / 8.0    # 1/sqrt(DK)

NCORES = 8
GRP = 4              # cores per batch group
T = (B * S) // NCORES  # 256 tokens per core
TC = T // 128        # 2 token chunks of 128
DC = D // 128        # 8 feature chunks
DFC = DF // 128      # 32 ffn feature chunks
KC = S // 128        # 8 key chunks per sequence
HP = H // 2          # 8 head pairs

REPLICA_GROUPS = [[0, 1, 2, 3], [4, 5, 6, 7]]

# merged K+V gather block: K^T [D rows=1024, T] then V [T,1024] viewed [1024, T]
KVROWS = DC * 128 + T * D // T  # 1024 + 1024 = 2048


def _sinusoidal_pe(seq_len, d_model):
    pos = np.arange(seq_len)[:, None]
    i = np.arange(0, d_model, 2)[None, :]
    angle = pos / np.power(10000.0, i / d_model)
    pe = np.zeros((seq_len, d_model), dtype=np.float32)
    pe[:, 0::2] = np.sin(angle)
    pe[:, 1::2] = np.cos(angle)
    return pe


# ----------------------------------------------------------------------------
# device program
# ----------------------------------------------------------------------------

def build_nc(use_mask: bool, num_layers: int = NL):
    nc = bacc.Bacc("TRN2", target_bir_lowering=False, debug=False,
                   num_devices=NCORES)

    # --- DRAM parameters (per core) ---
    srcT_d = nc.dram_tensor("srcT", [IN, T], BF16, kind="ExternalInput")
    peT_d = nc.dram_tensor("peT", [DC * 128, T], F32, kind="ExternalInput")
    wfc1_d = nc.dram_tensor("wfc1", [IN, 3 * D], BF16, kind="ExternalInput")
    # wfc2/wfc3 blocks: [blk, 128, 24ci, 256of]
    wfc2_d = nc.dram_tensor("wfc2", [24 * 128, 24, 128], BF16, kind="ExternalInput")
    wfc3_d = nc.dram_tensor("wfc3", [8 * 128, 24, 128], BF16, kind="ExternalInput")
    # per-layer weights
    wq_d = nc.dram_tensor("wq", [num_layers * 128, DC, D], BF16, kind="ExternalInput")
    wk_d = nc.dram_tensor("wk", [num_layers * 128, DC, D], BF16, kind="ExternalInput")
    wv_d = nc.dram_tensor("wv", [num_layers * 128, DC, D], BF16, kind="ExternalInput")
    # wo in 65-row head chunks (row 0 zero), 256-wide of-blocks:
    # [l, 4, 65, 16ci, 256of]
    wo_d = nc.dram_tensor("wo", [num_layers * 4 * 65, 16, 256], BF16, kind="ExternalInput")
    # w1 blocks: [l, blk8, 128, 8ci, 512of]; w2 blocks: [l, co8, 128, 32ci, 128of]
    w1_d = nc.dram_tensor("w1", [num_layers * 8 * 128, DC, 512], BF16, kind="ExternalInput")
    w2_d = nc.dram_tensor("w2", [num_layers * 8 * 128, DFC, 128], BF16, kind="ExternalInput")
    wout1_d = nc.dram_tensor("wout1", [128, DC, DR], BF16, kind="ExternalInput")
    wout2_d = nc.dram_tensor("wout2", [128, 2, 1], BF16, kind="ExternalInput")
    if use_mask:
        maskb_d = nc.dram_tensor("maskb", [KC * 128, T], F32, kind="ExternalInput")
    out_d = nc.dram_tensor("out", [1, T], F32, kind="ExternalOutput")

    with tile.TileContext(nc) as tc:
        import contextlib
        ctx = contextlib.ExitStack()
        with ctx:
            singles = ctx.enter_context(tc.tile_pool(name="singles", bufs=1))
            xpool = ctx.enter_context(tc.tile_pool(name="xpool", bufs=1))
            wstream = ctx.enter_context(tc.tile_pool(name="wstream", bufs=4))
            wqkv = ctx.enter_context(tc.tile_pool(name="wqkv", bufs=2))
            hpool = ctx.enter_context(tc.tile_pool(name="hpool", bufs=2))
            kvpool = ctx.enter_context(tc.tile_pool(name="kvpool", bufs=1))
            ppool = ctx.enter_context(tc.tile_pool(name="ppool", bufs=10))
            stats = ctx.enter_context(tc.tile_pool(name="stats", bufs=4))
            bcast = ctx.enter_context(tc.tile_pool(name="bcast", bufs=3))
            # mm tiles span TWO psum banks ([128, 2, 512] f32): GEMMs use
            # [:, 0, :T]; attention scores put the head pair at [:, 0, :T] and
            # [:, 1, :T] -- different banks, so the two accumulation groups
            # are legal, and ONE strided exp covers both heads.
            mm_ps = ctx.enter_context(tc.tile_pool(name="mm_ps", bufs=4, space="PSUM"))
            oe_ps = ctx.enter_context(tc.tile_pool(name="oe_ps", bufs=2, space="PSUM"))
            st_ps = ctx.enter_context(tc.tile_pool(name="st_ps", bufs=2, space="PSUM"))
            dram = ctx.enter_context(tc.tile_pool(name="dram", bufs=2, space="DRAM"))

            ones_bf = singles.tile([128, 1], BF16)
            nc.vector.memset(ones_bf[:], 1.0)
            ones_row = singles.tile([1, 128], F32)
            nc.vector.memset(ones_row[:], 1.0)
            eps_sb = singles.tile([1, 1], F32)
            nc.vector.memset(eps_sb[:], EPS)

            # residual stream, fp32 feature-major [128, DC, T]
            x_sb = xpool.tile([128, DC, T], F32)
            x2b = xpool.tile([128, DC, T], BF16)

            # V travels token-major with 65-wide head slots; slot col 0 is a
            # constant 1.0 (set once -- the GEMM only writes cols 1:65), so
            # the softmax denominator rides the PV matmul as PSUM row 0.
            vtb65 = xpool.tile([128, TC, H, 65], FP8)
            nc.vector.memset(vtb65[:, :, :, 0:1], 1.0)
            # gathered V: fp8 straight off the wire, converted to bf16 for PV
            vg65_8 = xpool.tile([128, GRP, TC, H, 65], FP8)
            vg65 = xpool.tile([128, GRP, TC, H, 65], BF16)

            if use_mask:
                maskb_sb = xpool.tile([128, KC, T], F32)
                nc.sync.dma_start(
                    maskb_sb[:], maskb_d.ap().rearrange("(c p) t -> p c t", p=128))

            def mmtile():
                return mm_ps.tile([128, 512], F32, tag="mm", name="mm")

            # ---------------- LayerNorm (feature axis) -> bf16 --------------
            # stats via bf16 ones-matmuls; squares on the scalar engine to
            # split the elementwise load between ACT and DVE.
            # rstd = exp(-0.5*ln(var+eps)) keeps ACT inside the
            # natural_log_exp table set (shared with the attention exp).
            def layer_norm(src_f32, dst_bf16):
                sum_ps = st_ps.tile([1, T], F32, tag="st")
                sq_ps = st_ps.tile([1, T], F32, tag="st")
                for c in range(DC):
                    xbc = bcast.tile([128, T], BF16, tag="xb", bufs=3, name="xbc")
                    xsqc = bcast.tile([128, T], BF16, tag="xsq", bufs=3,
                                      name="xsqc")
                    nc.vector.tensor_copy(xbc[:], src_f32[:, c, :])
                    nc.scalar.activation(out=xsqc[:], in_=src_f32[:, c, :],
                                         func=AF.Square, scale=1.0)
                    nc.tensor.matmul(sum_ps[:], ones_bf[:], xbc[:],
                                     start=(c == 0), stop=(c == DC - 1))
                    nc.tensor.matmul(sq_ps[:], ones_bf[:], xsqc[:],
                                     start=(c == 0), stop=(c == DC - 1))
                mean_r = stats.tile([1, T], F32)
                var_r = stats.tile([1, T], F32)
                rstd_r = stats.tile([1, T], F32)
                nmr_r = stats.tile([1, T], F32)
                nc.vector.tensor_scalar_mul(mean_r[:], sum_ps[:], 1.0 / D)
                nc.vector.tensor_mul(var_r[:], mean_r[:], mean_r[:])
                nc.vector.scalar_tensor_tensor(
                    var_r[:], sq_ps[:], 1.0 / D, var_r[:], ALU.mult, ALU.subtract)
                nc.scalar.activation(out=rstd_r[:], in_=var_r[:], func=AF.Ln,
                                     bias=eps_sb[:], scale=1.0)
                nc.scalar.activation(out=rstd_r[:], in_=rstd_r[:], func=AF.Exp,
                                     scale=-0.5)
                nc.vector.scalar_tensor_tensor(
                    nmr_r[:], mean_r[:], -1.0, rstd_r[:], ALU.mult, ALU.mult)
                # broadcast per-token stats to 128 partitions via fp32
                # ones-matmuls (PE is local; avoids the gpsimd queue hop)
                rstd_b = st_ps.tile([128, T], F32, tag="st", name="rstd_b")
                nmr_b = st_ps.tile([128, T], F32, tag="st", name="nmr_b")
                nc.tensor.matmul(rstd_b[:], ones_row[:], rstd_r[:],
                                 start=True, stop=True)
                nc.tensor.matmul(nmr_b[:], ones_row[:], nmr_r[:],
                                 start=True, stop=True)
                for c in range(DC):
                    t_f = bcast.tile([128, T], F32, tag="lnt")
                    nc.vector.tensor_mul(t_f[:], src_f32[:, c, :], rstd_b[:])
                    nc.vector.tensor_add(dst_bf16[:, c, :], t_f[:], nmr_b[:])

            # ------------- input MLP ---------------------------------------
            srcT_sb = singles.tile([IN, T], BF16)
            nc.sync.dma_start(srcT_sb[:], srcT_d.ap())
            wfc1_sb = wstream.tile([IN, 3 * D], BF16, tag="w")
            nc.sync.dma_start(wfc1_sb[:], wfc1_d.ap())

            h1 = hpool.tile([128, 24, T], BF16, tag="h")
            for co in range(24):
                pt = mmtile()
                nc.tensor.matmul(pt[:, :T], wfc1_sb[:, co * 128:(co + 1) * 128],
                                 srcT_sb[:], start=True, stop=True)
                nc.scalar.activation(out=h1[:, co, :], in_=pt[:, :T],
                                     func=AF.Relu, scale=1.0)

            h2 = hpool.tile([128, 24, T], BF16, tag="h")
            for co in range(24):
                wt = wstream.tile([128, 24, 128], BF16, tag="w")
                nc.sync.dma_start(wt[:], wfc2_d.ap()[co * 128:(co + 1) * 128])
                pt = mmtile()
                for ci in range(24):
                    nc.tensor.matmul(
                        pt[:, :T], wt[:, ci, :],
                        h1[:, ci, :], start=(ci == 0), stop=(ci == 23))
                nc.scalar.activation(out=h2[:, co, :], in_=pt[:, :T],
                                     func=AF.Relu, scale=1.0)

            peT_sb = hpool.tile([128, DC, T], F32, tag="h")
            nc.sync.dma_start(peT_sb[:], peT_d.ap().rearrange("(c p) t -> p c t", p=128))
            for co in range(DC):
                wt = wstream.tile([128, 24, 128], BF16, tag="w")
                nc.sync.dma_start(wt[:], wfc3_d.ap()[co * 128:(co + 1) * 128])
                pt = mmtile()
                for ci in range(24):
                    nc.tensor.matmul(
                        pt[:, :T], wt[:, ci, :],
                        h2[:, ci, :], start=(ci == 0), stop=(ci == 23))
                nc.vector.tensor_add(x_sb[:, co, :], pt[:, :T], peT_sb[:, co, :])

            # ------------- transformer layers ------------------------------
            for li in range(num_layers):
                layer_norm(x_sb, x2b)

                # K then V, then ONE merged gather; Q overlaps the collective.
                wk_sb = wqkv.tile([128, DC, D], BF16, tag="wqkv")
                nc.sync.dma_start(wk_sb[:], wk_d.ap()[li * 128:(li + 1) * 128])
                kTb = kvpool.tile([128, DC, T], FP8, tag="kT")
                for co in range(DC):
                    pt = mmtile()
                    for ci in range(DC):
                        nc.tensor.matmul(
                            pt[:, :T], wk_sb[:, ci, co * 128:(co + 1) * 128],
                            x2b[:, ci, :], start=(ci == 0), stop=(ci == DC - 1))
                    nc.vector.tensor_copy(kTb[:, co, :], pt[:, :T])

                wv_sb = wqkv.tile([128, DC, D], BF16, tag="wqkv")
                nc.sync.dma_start(wv_sb[:], wv_d.ap()[li * 128:(li + 1) * 128])
                for t in range(TC):
                    for dvb in range(2):
                        pt = mmtile()
                        for ci in range(DC):
                            nc.tensor.matmul(
                                pt[:], x2b[:, ci, t * 128:(t + 1) * 128],
                                wv_sb[:, ci, dvb * 512:(dvb + 1) * 512],
                                start=(ci == 0), stop=(ci == DC - 1))
                        nc.vector.tensor_copy(
                            vtb65[:, t, dvb * 8:(dvb + 1) * 8, 1:65],
                            pt[:].rearrange("p (h d) -> p h d", h=8))

                # merged K+V gather block (1-D):
                # [K^T 1024x256 | V-with-ones 256x1040]
                KSZ = D * T          # 262144
                VSZ = T * H * 65     # 266240
                kv_in = dram.tile([KSZ + VSZ], FP8, tag="kvin")
                nc.sync.dma_start(
                    kv_in[0:KSZ].rearrange("(c p t) -> p c t", p=128, t=T),
                    kTb[:])
                nc.sync.dma_start(
                    kv_in[KSZ:KSZ + VSZ].rearrange(
                        "(a p s) -> p a s", p=128, s=H * 65),
                    vtb65[:].rearrange("p a h s -> p a (h s)"))
                kv_g = dram.tile([GRP * (KSZ + VSZ)], FP8, tag="kvg")
                nc.gpsimd.collective_compute(
                    "AllGather", ALU.bypass, replica_groups=REPLICA_GROUPS,
                    ins=[kv_in[:].opt()], outs=[kv_g[:].opt()])

                # Q while the collective is in flight
                wq_sb = wqkv.tile([128, DC, D], BF16, tag="wqkv")
                nc.sync.dma_start(wq_sb[:], wq_d.ap()[li * 128:(li + 1) * 128])
                qTb = kvpool.tile([128, DC, T], FP8, tag="qT")
                for co in range(DC):
                    pt = mmtile()
                    for ci in range(DC):
                        nc.tensor.matmul(
                            pt[:, :T], wq_sb[:, ci, co * 128:(co + 1) * 128],
                            x2b[:, ci, :], start=(ci == 0), stop=(ci == DC - 1))
                    nc.vector.tensor_copy(qTb[:, co, :], pt[:, :T])

                # reload gathered K (feature-major) and V (65-slot layout)
                kTg = kvpool.tile([128, DC, GRP, T], FP8, tag="kTg")
                for r in range(GRP):
                    base = r * (KSZ + VSZ)
                    nc.sync.dma_start(
                        kTg[:, :, r, :],
                        kv_g[base:base + KSZ].rearrange(
                            "(c p t) -> p c t", p=128, t=T))
                    nc.sync.dma_start(
                        vg65_8[:, r, :, :, :].rearrange("p a h s -> p a (h s)"),
                        kv_g[base + KSZ:base + KSZ + VSZ].rearrange(
                            "(a p s) -> p a s", p=128, s=H * 65))
                    nc.vector.tensor_copy(
                        vg65[:, r, :, :, :].rearrange("p a h s -> p a (h s)"),
                        vg65_8[:, r, :, :, :].rearrange("p a h s -> p a (h s)"))

                # attention: head pairs share one [128,512] score bank ->
                # one exp / one recip / one broadcast / one normalize per pair.
                # denominator rides PV as PSUM row 0 (vg65 slot col 0 == 1).
                o_allb = kvpool.tile([65, HP, 512], BF16, tag="oall")
                for h in range(H):
                    bp = (h % 2) * 64
                    cf = h // 2
                    oe = oe_ps.tile([65, T], F32, tag="oe")
                    for c in range(KC):
                        sp = mmtile()
                        nc.tensor.matmul(
                            sp[:, :T],
                            kTg[bp:bp + 64, cf, c // 2,
                                (c % 2) * 128:(c % 2) * 128 + 128],
                            qTb[bp:bp + 64, cf, :], start=True, stop=True)
                        pb = ppool.tile([128, T], BF16, tag="p")
                        if use_mask:
                            nc.vector.tensor_add(
                                sp[:, :T], sp[:, :T], maskb_sb[:, c, :])
                        nc.scalar.activation(out=pb[:], in_=sp[:, :T],
                                             func=AF.Exp, scale=SCALE)
                        nc.tensor.matmul(
                            oe[:], vg65[:, c // 2, c % 2, h, :], pb[:],
                            start=(c == 0), stop=(c == KC - 1))
                    recip = stats.tile([1, T], F32, tag="recip")
                    nc.vector.reciprocal_approx_fast(recip[:], oe[0:1, :])
                    # broadcast 1/denom via a PE ones-matmul (gpsimd
                    # partition_broadcast hops are slow); bounce through SBUF
                    # because tensor_mul cannot take two PSUM operands.
                    rb_ps = st_ps.tile([65, T], F32, tag="st", name="rb_ps")
                    nc.tensor.matmul(rb_ps[:], ones_row[:, :65], recip[:],
                                     start=True, stop=True)
                    rb = bcast.tile([65, T], F32, tag="rb")
                    nc.vector.tensor_copy(rb[:], rb_ps[:])
                    nc.vector.tensor_mul(
                        o_allb[:, h // 2, (h % 2) * T:(h % 2) * T + T],
                        oe[:], rb[:])

                # output projection (65-row chunks, row 0 zero) + residual
                for blk in range(4):
                    wt = wstream.tile([65, 16, 256], BF16, tag="w")
                    nc.sync.dma_start(wt[:], wo_d.ap()[
                        (li * 4 + blk) * 65:(li * 4 + blk + 1) * 65])
                    for co2 in range(2):
                        co = blk * 2 + co2
                        pt = mmtile()
                        for ci in range(16):
                            nc.tensor.matmul(
                                pt[:, :T], wt[:, ci, co2 * 128:(co2 + 1) * 128],
                                o_allb[:, ci // 2, (ci % 2) * T:(ci % 2) * T + T],
                                start=(ci == 0), stop=(ci == 15))
                        nc.vector.tensor_add(
                            x_sb[:, co, :], x_sb[:, co, :], pt[:, :T])

                # FFN
                layer_norm(x_sb, x2b)
                hT = hpool.tile([128, DFC, T], BF16, tag="h")
                for blk in range(8):  # 512 hidden features per block
                    wt = wstream.tile([128, DC, 512], BF16, tag="w")
                    nc.sync.dma_start(wt[:], w1_d.ap()[
                        (li * 8 + blk) * 128:(li * 8 + blk + 1) * 128])
                    for co2 in range(4):
                        co = blk * 4 + co2
                        pt = mmtile()
                        for ci in range(DC):
                            nc.tensor.matmul(
                                pt[:, :T], wt[:, ci, co2 * 128:(co2 + 1) * 128],
                                x2b[:, ci, :], start=(ci == 0), stop=(ci == DC - 1))
                        nc.scalar.activation(out=hT[:, co, :], in_=pt[:, :T],
                                             func=AF.Gelu, scale=1.0)
                for co in range(DC):
                    wt = wstream.tile([128, DFC, 128], BF16, tag="w")
                    nc.sync.dma_start(wt[:], w2_d.ap()[
                        (li * 8 + co) * 128:(li * 8 + co + 1) * 128])
                    pt = mmtile()
                    for ci in range(DFC):
                        nc.tensor.matmul(pt[:, :T], wt[:, ci, :], hT[:, ci, :],
                                         start=(ci == 0), stop=(ci == DFC - 1))
                    nc.vector.tensor_add(x_sb[:, co, :], x_sb[:, co, :], pt[:, :T])

            # ------------- final LN + head ---------------------------------
            layer_norm(x_sb, x2b)
            wout1_sb = wstream.tile([128, DC, DR], BF16, tag="w")
            nc.sync.dma_start(wout1_sb[:], wout1_d.ap())
            wout2_sb = wstream.tile([128, 2, 1], BF16, tag="w2")
            nc.sync.dma_start(wout2_sb[:], wout2_d.ap())
            h3 = hpool.tile([128, 2, T], BF16, tag="h3")
            for co in range(2):
                pt = mmtile()
                for ci in range(DC):
                    nc.tensor.matmul(
                        pt[:, :T], wout1_sb[:, ci, co * 128:(co + 1) * 128],
                        x2b[:, ci, :], start=(ci == 0), stop=(ci == DC - 1))
                nc.vector.tensor_copy(h3[:, co, :], pt[:, :T])
            fin = st_ps.tile([1, T], F32, tag="st")
            for ci in range(2):
                nc.tensor.matmul(fin[:], wout2_sb[:, ci, :], h3[:, ci, :],
                                 start=(ci == 0), stop=(ci == 1))
            fin_sb = stats.tile([1, T], F32, tag="fin")
            nc.vector.tensor_copy(fin_sb[:], fin[:])
            nc.sync.dma_start(out_d.ap(), fin_sb[:])

    nc.compile()
    return nc


# ----------------------------------------------------------------------------
# host side
# ----------------------------------------------------------------------------

_cache = {}


def _get_nc(use_mask, num_layers=NL):
    key = (use_mask, num_layers)
    if key not in _cache:
        _cache[key] = build_nc(use_mask, num_layers)
    return _cache[key]


def _bf(a):
    return np.ascontiguousarray(a).astype(ml_dtypes.bfloat16)


def prep_inputs(inputs, num_layers=NL):
    """Host-side prep: fold LN gains into the following matmuls, pre-arrange
    weights into contiguous DMA blocks, shard tokens across cores."""
    f = {k: np.asarray(v) for k, v in inputs.items()}
    src = f["src"].astype(np.float32)            # [B,S,IN]
    mask = np.asarray(f["mask"])
    use_mask = not bool((mask == 1).all())

    ln1_g, ln2_g, lnf_g = f["ln1_g"], f["ln2_g"], f["lnf_g"]

    # setup_inputs always uses zero biases / LN b; the device program carries
    # no bias adds, so require that here (fail loudly otherwise).
    for name in ("ln1_b", "ln2_b", "lnf_b", "bfc1", "bfc2", "bfc3", "bo",
                 "b1", "b2", "bout1", "bout2"):
        if np.abs(f[name]).max() != 0.0:
            raise NotImplementedError(f"nonzero bias {name} not supported")

    nl = num_layers
    wq = (f["Wq"] * ln1_g[:, :, None])[:nl]      # [nl,D,D]
    wk = (f["Wk"] * ln1_g[:, :, None])[:nl]
    wv = (f["Wv"] * ln1_g[:, :, None])[:nl]
    wo = f["Wo"][:nl]
    w1 = (f["W1"] * ln2_g[:, :, None])[:nl]      # [nl,D,DF]
    w2 = f["W2"][:nl]                            # [nl,DF,D]
    wout1 = f["Wout1"] * lnf_g[:, None]          # [D,DR]
    wout2 = f["Wout2"]                           # [DR,1]

    def pcf(w):  # [L,IN_,OF] -> [L,128,IN_/128,OF]
        L, i, o = w.shape
        return w.reshape(L, i // 128, 128, o).transpose(0, 2, 1, 3)

    wq_h, wk_h, wv_h = (
        _bf(pcf(w)).reshape(num_layers * 128, DC, D) for w in (wq, wk, wv))
    # wo 65-row head chunks (row 0 zero), of-blocks: [L, 4, 65, 16ci, 256]
    wo_r = wo.reshape(num_layers, 16, 64, 4, 256)
    wo65 = np.zeros((num_layers, 16, 65, 4, 256), dtype=np.float32)
    wo65[:, :, 1:, :, :] = wo_r
    wo_h = _bf(wo65.transpose(0, 3, 2, 1, 4).reshape(num_layers * 4 * 65, 16, 256))
    # w1 blocks [L, blk8, 128, 8ci, 512of]
    w1_h = _bf(w1.reshape(num_layers, DC, 128, 8, 512).transpose(0, 3, 2, 1, 4).reshape(num_layers * 8 * 128, DC, 512))
    # w2 blocks [L, co8, 128, 32ci, 128of]
    w2_h = _bf(w2.reshape(num_layers, DFC, 128, DC, 128).transpose(0, 3, 2, 1, 4).reshape(num_layers * 8 * 128, DFC, 128))
    wfc1_h = _bf(f["Wfc1"])                      # [64, 3072]
    # wfc2 blocks [12, 128, 24ci, 256of]
    wfc2_h = _bf(f["Wfc2"].reshape(24, 128, 24, 128).transpose(2, 1, 0, 3)
                 .reshape(24 * 128, 24, 128))
    wfc3_h = _bf(f["Wfc3"].reshape(24, 128, 8, 128).transpose(2, 1, 0, 3)
                 .reshape(8 * 128, 24, 128))
    wout1_h = _bf(wout1.reshape(DC, 128, DR).transpose(1, 0, 2))  # [128,8,256]
    wout2_h = _bf(wout2.reshape(2, 128, 1).transpose(1, 0, 2))    # [128,2,1]

    pe = _sinusoidal_pe(S, D)                    # [S,D]

    in_maps = []
    for core in range(NCORES):
        b = core // GRP
        t0 = (core % GRP) * T
        srcT = _bf(src[b, t0:t0 + T, :].T)       # [64, T]
        peT = np.ascontiguousarray(
            pe[t0:t0 + T, :].T).astype(np.float32)
        m = {
            "srcT": srcT, "peT": peT,
            "wfc1": wfc1_h, "wfc2": wfc2_h, "wfc3": wfc3_h,
            "wq": wq_h, "wk": wk_h, "wv": wv_h, "wo": wo_h,
            "w1": w1_h, "w2": w2_h,
            "wout1": wout1_h, "wout2": wout2_h,
        }
        if use_mask:
            mb = np.where(mask[b, t0:t0 + T, :] == 0, -8e9, 0.0).astype(np.float32)
            m["maskb"] = np.ascontiguousarray(mb.T)
        in_maps.append(m)
    return in_maps, use_mask


def kernel(**inputs):
    in_maps, use_mask = prep_inputs(inputs)
    nc = _get_nc(use_mask)
    res = bass_utils.run_bass_kernel_spmd(
        nc, in_maps, core_ids=list(range(NCORES)))
    out = np.concatenate(
        [res.results[i]["out"].reshape(-1) for i in range(NCORES)])
    return out.reshape(B, S, 1).astype(np.float32)


# revision 38
# speedup vs baseline: 1.1594x; 1.1594x over previous
"""Trainium2 Bass kernel for nn_BERT_61873298866553.

6-layer pre-norm BERT encoder (B=2, S=1024, D=1024, H=16, DF=4096) with a
3-layer input MLP and a 2-layer output head.

Distribution: 8-way sequence sharding (core i owns batch i//4, tokens
(i%4)*256..+256).  Everything is token-local except attention K/V, which is
all-gathered per layer inside the two 4-core batch groups
(replica_groups=[[0..3],[4..7]]) as ONE merged K+V collective.

On-device layout: activations are feature-major (features on SBUF
partitions, tokens on the free axis), so every linear is
out^T[of,t] = sum_ci W[ci,of]^T @ x^T[ci,t] with W chunks stationary.
GEMMs run in bf16 with fp32 PSUM accumulation; the residual stream,
LayerNorm and softmax statistics stay fp32.  LN reduces over the feature
(partition) axis with fp32 ones-matmuls; per-token stats are broadcast via
gpsimd partition_broadcast (which requires base-partition-0 inputs on HW).

Attention: heads processed in pairs (h, h+1) sharing one [128,512] PSUM
score bank -> one exp per pair.  V is stored in a persistent [128,KC,H,65]
SBUF tile whose slot column 0 is a constant 1.0 (memset once), so the
softmax denominator rides the PV matmul as PSUM row 0 -- no separate
denominator matmuls.  Per pair: one reciprocal_approx_fast on [1,512],
one partition_broadcast, one tensor_mul.  Wo is consumed as 16 chunks of
65 rows whose row 0 is zero (kills the garbage dn*recip row).
"""

import sys

if "/opt/trn_rl_repo" not in sys.path:
    sys.path.insert(0, "/opt/trn_rl_repo")

import numpy as np
import ml_dtypes

import concourse.bass as bass
import concourse.tile as tile
import concourse.mybir as mybir
from concourse import bacc
from concourse import bass_utils

F32 = mybir.dt.float32
BF16 = mybir.dt.bfloat16
FP8 = mybir.dt.float8e4
AF = mybir.ActivationFunctionType
ALU = mybir.AluOpType

# Model dims (fixed by the problem).
B, S, IN = 2, 1024, 64
D, H, NL, DF = 1024, 16, 6, 4096
DK = D // H          # 64
DR = D // 4          # 256
EPS = 1e-5
SCALE = 1.0 / 8.0    # 1/sqrt(DK)

NCORES = 8
GRP = 4              # cores per batch group
T = (B * S) // NCORES  # 256 tokens per core
TC = T // 128        # 2 token chunks of 128
DC = D // 128        # 8 feature chunks
DFC = DF // 128      # 32 ffn feature chunks
KC = S // 128        # 8 key chunks per sequence
HP = H // 2          # 8 head pairs

REPLICA_GROUPS = [[0, 1, 2, 3], [4, 5, 6, 7]]

# merged K+V gather block: K^T [D rows=1024, T] then V [T,1024] viewed [1024, T]
KVROWS = DC * 128 + T * D // T  # 1024 + 1024 = 2048


def _sinusoidal_pe(seq_len, d_model):
    pos = np.arange(seq_len)[:, None]
    i = np.arange(0, d_model, 2)[None, :]
    angle = pos / np.power(10000.0, i / d_model)
    pe = np.zeros((seq_len, d_model), dtype=np.float32)
    pe[:, 0::2] = np.sin(angle)
    pe[:, 1::2] = np.cos(angle)
    return pe


# ----------------------------------------------------------------------------
# device program
# ----------------------------------------------------------------------------

def build_nc(use_mask: bool, num_layers: int = NL):
    nc = bacc.Bacc("TRN2", target_bir_lowering=False, debug=False,
                   num_devices=NCORES)

    # --- DRAM parameters (per core) ---
    srcT_d = nc.dram_tensor("srcT", [IN, T], BF16, kind="ExternalInput")
    peT_d = nc.dram_tensor("peT", [DC * 128, T], F32, kind="ExternalInput")
    wfc1_d = nc.dram_tensor("wfc1", [IN, 3 * D], BF16, kind="ExternalInput")
    # wfc2/wfc3 blocks: [blk, 128, 24ci, 256of]
    wfc2_d = nc.dram_tensor("wfc2", [24 * 128, 24, 128], BF16, kind="ExternalInput")
    wfc3_d = nc.dram_tensor("wfc3", [8 * 128, 24, 128], BF16, kind="ExternalInput")
    # per-layer weights
    wq_d = nc.dram_tensor("wq", [num_layers * 128, DC, D], BF16, kind="ExternalInput")
    wk_d = nc.dram_tensor("wk", [num_layers * 128, DC, D], BF16, kind="ExternalInput")
    wv_d = nc.dram_tensor("wv", [num_layers * 128, DC, D], BF16, kind="ExternalInput")
    # wo in 65-row head chunks (row 0 zero), 256-wide of-blocks:
    # [l, 4, 65, 16ci, 256of]
    wo_d = nc.dram_tensor("wo", [num_layers * 4 * 65, 16, 256], BF16, kind="ExternalInput")
    # w1 blocks: [l, blk8, 128, 8ci, 512of]; w2 blocks: [l, co8, 128, 32ci, 128of]
    w1_d = nc.dram_tensor("w1", [num_layers * 8 * 128, DC, 512], BF16, kind="ExternalInput")
    w2_d = nc.dram_tensor("w2", [num_layers * 8 * 128, DFC, 128], BF16, kind="ExternalInput")
    wout1_d = nc.dram_tensor("wout1", [128, DC, DR], BF16, kind="ExternalInput")
    wout2_d = nc.dram_tensor("wout2", [128, 2, 1], BF16, kind="ExternalInput")
    if use_mask:
        maskb_d = nc.dram_tensor("maskb", [KC * 128, T], F32, kind="ExternalInput")
    out_d = nc.dram_tensor("out", [1, T], F32, kind="ExternalOutput")

    with tile.TileContext(nc) as tc:
        import contextlib
        ctx = contextlib.ExitStack()
        with ctx:
            singles = ctx.enter_context(tc.tile_pool(name="singles", bufs=1))
            xpool = ctx.enter_context(tc.tile_pool(name="xpool", bufs=1))
            wstream = ctx.enter_context(tc.tile_pool(name="wstream", bufs=4))
            wqkv = ctx.enter_context(tc.tile_pool(name="wqkv", bufs=2))
            hpool = ctx.enter_context(tc.tile_pool(name="hpool", bufs=2))
            kvpool = ctx.enter_context(tc.tile_pool(name="kvpool", bufs=1))
            ppool = ctx.enter_context(tc.tile_pool(name="ppool", bufs=10))
            stats = ctx.enter_context(tc.tile_pool(name="stats", bufs=4))
            bcast = ctx.enter_context(tc.tile_pool(name="bcast", bufs=3))
            # mm tiles span TWO psum banks ([128, 2, 512] f32): GEMMs use
            # [:, 0, :T]; attention scores put the head pair at [:, 0, :T] and
            # [:, 1, :T] -- different banks, so the two accumulation groups
            # are legal, and ONE strided exp covers both heads.
            mm_ps = ctx.enter_context(tc.tile_pool(name="mm_ps", bufs=4, space="PSUM"))
            oe_ps = ctx.enter_context(tc.tile_pool(name="oe_ps", bufs=2, space="PSUM"))
            st_ps = ctx.enter_context(tc.tile_pool(name="st_ps", bufs=2, space="PSUM"))
            dram = ctx.enter_context(tc.tile_pool(name="dram", bufs=2, space="DRAM"))

            ones_bf = singles.tile([128, 1], BF16)
            nc.vector.memset(ones_bf[:], 1.0)
            ones_row = singles.tile([1, 128], F32)
            nc.vector.memset(ones_row[:], 1.0)
            eps_sb = singles.tile([1, 1], F32)
            nc.vector.memset(eps_sb[:], EPS)

            # residual stream, fp32 feature-major [128, DC, T]
            x_sb = xpool.tile([128, DC, T], F32)
            x2b = xpool.tile([128, DC, T], BF16)

            # V travels token-major with 65-wide head slots; slot col 0 is a
            # constant 1.0 (set once -- the GEMM only writes cols 1:65), so
            # the softmax denominator rides the PV matmul as PSUM row 0.
            vtb65 = xpool.tile([128, TC, H, 65], FP8)
            nc.vector.memset(vtb65[:, :, :, 0:1], 1.0)
            # gathered V: fp8 straight off the wire, converted to bf16 for PV
            vg65_8 = xpool.tile([128, GRP, TC, H, 65], FP8)
            vg65 = xpool.tile([128, GRP, TC, H, 65], BF16)

            if use_mask:
                maskb_sb = xpool.tile([128, KC, T], F32)
                nc.sync.dma_start(
                    maskb_sb[:], maskb_d.ap().rearrange("(c p) t -> p c t", p=128))

            def mmtile():
                return mm_ps.tile([128, 512], F32, tag="mm", name="mm")

            # ---------------- LayerNorm (feature axis) -> bf16 --------------
            # stats via bf16 ones-matmuls; squares on the scalar engine to
            # split the elementwise load between ACT and DVE.
            # rstd = exp(-0.5*ln(var+eps)) keeps ACT inside the
            # natural_log_exp table set (shared with the attention exp).
            def layer_norm(src_f32, dst_bf16):
                sum_ps = st_ps.tile([1, T], F32, tag="st")
                sq_ps = st_ps.tile([1, T], F32, tag="st")
                for c in range(DC):
                    xbc = bcast.tile([128, T], BF16, tag="xb", bufs=3, name="xbc")
                    xsqc = bcast.tile([128, T], BF16, tag="xsq", bufs=3,
                                      name="xsqc")
                    nc.vector.tensor_copy(xbc[:], src_f32[:, c, :])
                    nc.scalar.activation(out=xsqc[:], in_=src_f32[:, c, :],
                                         func=AF.Square, scale=1.0)
                    nc.tensor.matmul(sum_ps[:], ones_bf[:], xbc[:],
                                     start=(c == 0), stop=(c == DC - 1))
                    nc.tensor.matmul(sq_ps[:], ones_bf[:], xsqc[:],
                                     start=(c == 0), stop=(c == DC - 1))
                mean_r = stats.tile([1, T], F32)
                var_r = stats.tile([1, T], F32)
                rstd_r = stats.tile([1, T], F32)
                nmr_r = stats.tile([1, T], F32)
                nc.vector.tensor_scalar_mul(mean_r[:], sum_ps[:], 1.0 / D)
                nc.vector.tensor_mul(var_r[:], mean_r[:], mean_r[:])
                nc.vector.scalar_tensor_tensor(
                    var_r[:], sq_ps[:], 1.0 / D, var_r[:], ALU.mult, ALU.subtract)
                nc.scalar.activation(out=rstd_r[:], in_=var_r[:], func=AF.Ln,
                                     bias=eps_sb[:], scale=1.0)
                nc.scalar.activation(out=rstd_r[:], in_=rstd_r[:], func=AF.Exp,
                                     scale=-0.5)
                nc.vector.scalar_tensor_tensor(
                    nmr_r[:], mean_r[:], -1.0, rstd_r[:], ALU.mult, ALU.mult)
                # broadcast per-token stats to 128 partitions via fp32
                # ones-matmuls (PE is local; avoids the gpsimd queue hop)
                rstd_b = st_ps.tile([128, T], F32, tag="st", name="rstd_b")
                nmr_b = st_ps.tile([128, T], F32, tag="st", name="nmr_b")
                nc.tensor.matmul(rstd_b[:], ones_row[:], rstd_r[:],
                                 start=True, stop=True)
                nc.tensor.matmul(nmr_b[:], ones_row[:], nmr_r[:],
                                 start=True, stop=True)
                for c in range(DC):
                    t_f = bcast.tile([128, T], F32, tag="lnt")
                    nc.vector.tensor_mul(t_f[:], src_f32[:, c, :], rstd_b[:])
                    nc.vector.tensor_add(dst_bf16[:, c, :], t_f[:], nmr_b[:])

            # ------------- input MLP ---------------------------------------
            srcT_sb = singles.tile([IN, T], BF16)
            nc.sync.dma_start(srcT_sb[:], srcT_d.ap())
            wfc1_sb = wstream.tile([IN, 3 * D], BF16, tag="w")
            nc.sync.dma_start(wfc1_sb[:], wfc1_d.ap())

            h1 = hpool.tile([128, 24, T], BF16, tag="h")
            for co in range(24):
                pt = mmtile()
                nc.tensor.matmul(pt[:, :T], wfc1_sb[:, co * 128:(co + 1) * 128],
                                 srcT_sb[:], start=True, stop=True)
                nc.scalar.activation(out=h1[:, co, :], in_=pt[:, :T],
                                     func=AF.Relu, scale=1.0)

            h2 = hpool.tile([128, 24, T], BF16, tag="h")
            for co in range(24):
                wt = wstream.tile([128, 24, 128], BF16, tag="w")
                nc.sync.dma_start(wt[:], wfc2_d.ap()[co * 128:(co + 1) * 128])
                pt = mmtile()
                for ci in range(24):
                    nc.tensor.matmul(
                        pt[:, :T], wt[:, ci, :],
                        h1[:, ci, :], start=(ci == 0), stop=(ci == 23))
                nc.scalar.activation(out=h2[:, co, :], in_=pt[:, :T],
                                     func=AF.Relu, scale=1.0)

            peT_sb = hpool.tile([128, DC, T], F32, tag="h")
            nc.sync.dma_start(peT_sb[:], peT_d.ap().rearrange("(c p) t -> p c t", p=128))
            for co in range(DC):
                wt = wstream.tile([128, 24, 128], BF16, tag="w")
                nc.sync.dma_start(wt[:], wfc3_d.ap()[co * 128:(co + 1) * 128])
                pt = mmtile()
                for ci in range(24):
                    nc.tensor.matmul(
                        pt[:, :T], wt[:, ci, :],
                        h2[:, ci, :], start=(ci == 0), stop=(ci == 23))
                nc.vector.tensor_add(x_sb[:, co, :], pt[:, :T], peT_sb[:, co, :])

            # ------------- transformer layers ------------------------------
            for li in range(num_layers):
                layer_norm(x_sb, x2b)

                # K then V, then ONE merged gather; Q overlaps the collective.
                wk_sb = wqkv.tile([128, DC, D], BF16, tag="wqkv")
                nc.sync.dma_start(wk_sb[:], wk_d.ap()[li * 128:(li + 1) * 128])
                kTb = kvpool.tile([128, DC, T], FP8, tag="kT")
                for co in range(DC):
                    pt = mmtile()
                    for ci in range(DC):
                        nc.tensor.matmul(
                            pt[:, :T], wk_sb[:, ci, co * 128:(co + 1) * 128],
                            x2b[:, ci, :], start=(ci == 0), stop=(ci == DC - 1))
                    nc.vector.tensor_copy(kTb[:, co, :], pt[:, :T])

                wv_sb = wqkv.tile([128, DC, D], BF16, tag="wqkv")
                nc.sync.dma_start(wv_sb[:], wv_d.ap()[li * 128:(li + 1) * 128])
                for t in range(TC):
                    for dvb in range(2):
                        pt = mmtile()
                        for ci in range(DC):
                            nc.tensor.matmul(
                                pt[:], x2b[:, ci, t * 128:(t + 1) * 128],
                                wv_sb[:, ci, dvb * 512:(dvb + 1) * 512],
                                start=(ci == 0), stop=(ci == DC - 1))
                        nc.vector.tensor_copy(
                            vtb65[:, t, dvb * 8:(dvb + 1) * 8, 1:65],
                            pt[:].rearrange("p (h d) -> p h d", h=8))

                # merged K+V gather block (1-D):
                # [K^T 1024x256 | V-with-ones 256x1040]
                KSZ = D * T          # 262144
                VSZ = T * H * 65     # 266240
                kv_in = dram.tile([KSZ + VSZ], FP8, tag="kvin")
                nc.sync.dma_start(
                    kv_in[0:KSZ].rearrange("(c p t) -> p c t", p=128, t=T),
                    kTb[:])
                nc.sync.dma_start(
                    kv_in[KSZ:KSZ + VSZ].rearrange(
                        "(a p s) -> p a s", p=128, s=H * 65),
                    vtb65[:].rearrange("p a h s -> p a (h s)"))
                kv_g = dram.tile([GRP * (KSZ + VSZ)], FP8, tag="kvg")
                nc.gpsimd.collective_compute(
                    "AllGather", ALU.bypass, replica_groups=REPLICA_GROUPS,
                    ins=[kv_in[:].opt()], outs=[kv_g[:].opt()])

                # Q while the collective is in flight
                wq_sb = wqkv.tile([128, DC, D], BF16, tag="wqkv")
                nc.sync.dma_start(wq_sb[:], wq_d.ap()[li * 128:(li + 1) * 128])
                qTb = kvpool.tile([128, DC, T], FP8, tag="qT")
                for co in range(DC):
                    pt = mmtile()
                    for ci in range(DC):
                        nc.tensor.matmul(
                            pt[:, :T], wq_sb[:, ci, co * 128:(co + 1) * 128],
                            x2b[:, ci, :], start=(ci == 0), stop=(ci == DC - 1))
                    nc.vector.tensor_copy(qTb[:, co, :], pt[:, :T])

                # reload gathered K (feature-major) and V (65-slot layout)
                kTg = kvpool.tile([128, DC, GRP, T], FP8, tag="kTg")
                for r in range(GRP):
                    base = r * (KSZ + VSZ)
                    nc.sync.dma_start(
                        kTg[:, :, r, :],
                        kv_g[base:base + KSZ].rearrange(
                            "(c p t) -> p c t", p=128, t=T))
                    nc.sync.dma_start(
                        vg65_8[:, r, :, :, :].rearrange("p a h s -> p a (h s)"),
                        kv_g[base + KSZ:base + KSZ + VSZ].rearrange(
                            "(a p s) -> p a s", p=128, s=H * 65))
                    nc.vector.tensor_copy(
                        vg65[:, r, :, :, :].rearrange("p a h s -> p a (h s)"),
                        vg65_8[:, r, :, :, :].rearrange("p a h s -> p a (h s)"))

                # attention: head pairs share one [128,512] score bank ->
                # one exp / one recip / one broadcast / one normalize per pair.
                # denominator rides PV as PSUM row 0 (vg65 slot col 0 == 1).
                o_allb = kvpool.tile([65, HP, 512], BF16, tag="oall")
                for h in range(H):
                    bp = (h % 2) * 64
                    cf = h // 2
                    oe = oe_ps.tile([65, T], F32, tag="oe")
                    for c in range(KC):
                        sp = mmtile()
                        nc.tensor.matmul(
                            sp[:, :T],
                            kTg[bp:bp + 64, cf, c // 2,
                                (c % 2) * 128:(c % 2) * 128 + 128],
                            qTb[bp:bp + 64, cf, :], start=True, stop=True)
                        pb = ppool.tile([128, T], BF16, tag="p")
                        if use_mask:
                            nc.vector.tensor_add(
                                sp[:, :T], sp[:, :T], maskb_sb[:, c, :])
                        nc.scalar.activation(out=pb[:], in_=sp[:, :T],
                                             func=AF.Exp, scale=SCALE)
                        nc.tensor.matmul(
                            oe[:], vg65[:, c // 2, c % 2, h, :], pb[:],
                            start=(c == 0), stop=(c == KC - 1))
                    recip = stats.tile([1, T], F32, tag="recip")
                    nc.vector.reciprocal_approx_fast(recip[:], oe[0:1, :])
                    # NOTE: PE ones-matmul broadcast (as in layer_norm) is not
                    # usable here -- rb would land in PSUM and tensor_mul
                    # cannot take two PSUM operands (birverifier rejects it).
                    rb = bcast.tile([65, T], F32, tag="rb")
                    nc.gpsimd.partition_broadcast(rb[:], recip[:])
                    nc.vector.tensor_mul(
                        o_allb[:, h // 2, (h % 2) * T:(h % 2) * T + T],
                        oe[:], rb[:])

                # output projection (65-row chunks, row 0 zero) + residual
                for blk in range(4):
                    wt = wstream.tile([65, 16, 256], BF16, tag="w")
                    nc.sync.dma_start(wt[:], wo_d.ap()[
                        (li * 4 + blk) * 65:(li * 4 + blk + 1) * 65])
                    for co2 in range(2):
                        co = blk * 2 + co2
                        pt = mmtile()
                        for ci in range(16):
                            nc.tensor.matmul(
                                pt[:, :T], wt[:, ci, co2 * 128:(co2 + 1) * 128],
                                o_allb[:, ci // 2, (ci % 2) * T:(ci % 2) * T + T],
                                start=(ci == 0), stop=(ci == 15))
                        nc.vector.tensor_add(
                            x_sb[:, co, :], x_sb[:, co, :], pt[:, :T])

                # FFN
                layer_norm(x_sb, x2b)
                hT = hpool.tile([128, DFC, T], BF16, tag="h")
                for blk in range(8):  # 512 hidden features per block
                    wt = wstream.tile([128, DC, 512], BF16, tag="w")
                    nc.sync.dma_start(wt[:], w1_d.ap()[
                        (li * 8 + blk) * 128:(li * 8 + blk + 1) * 128])
                    for co2 in range(4):
                        co = blk * 4 + co2
                        pt = mmtile()
                        for ci in range(DC):
                            nc.tensor.matmul(
                                pt[:, :T], wt[:, ci, co2 * 128:(co2 + 1) * 128],
                                x2b[:, ci, :], start=(ci == 0), stop=(ci == DC - 1))
                        nc.scalar.activation(out=hT[:, co, :], in_=pt[:, :T],
                                             func=AF.Gelu, scale=1.0)
                for co in range(DC):
                    wt = wstream.tile([128, DFC, 128], BF16, tag="w")
                    nc.sync.dma_start(wt[:], w2_d.ap()[
                        (li * 8 + co) * 128:(li * 8 + co + 1) * 128])
                    pt = mmtile()
                    for ci in range(DFC):
                        nc.tensor.matmul(pt[:, :T], wt[:, ci, :], hT[:, ci, :],
                                         start=(ci == 0), stop=(ci == DFC - 1))
                    nc.vector.tensor_add(x_sb[:, co, :], x_sb[:, co, :], pt[:, :T])

            # ------------- final LN + head ---------------------------------
            layer_norm(x_sb, x2b)
            wout1_sb = wstream.tile([128, DC, DR], BF16, tag="w")
            nc.sync.dma_start(wout1_sb[:], wout1_d.ap())
            wout2_sb = wstream.tile([128, 2, 1], BF16, tag="w2")
            nc.sync.dma_start(wout2_sb[:], wout2_d.ap())
            h3 = hpool.tile([128, 2, T], BF16, tag="h3")
            for co in range(2):
                pt = mmtile()
                for ci in range(DC):
                    nc.tensor.matmul(
                        pt[:, :T], wout1_sb[:, ci, co * 128:(co + 1) * 128],
                        x2b[:, ci, :], start=(ci == 0), stop=(ci == DC - 1))
                nc.vector.tensor_copy(h3[:, co, :], pt[:, :T])
            fin = st_ps.tile([1, T], F32, tag="st")
            for ci in range(2):
                nc.tensor.matmul(fin[:], wout2_sb[:, ci, :], h3[:, ci, :],
                                 start=(ci == 0), stop=(ci == 1))
            fin_sb = stats.tile([1, T], F32, tag="fin")
            nc.vector.tensor_copy(fin_sb[:], fin[:])
            nc.sync.dma_start(out_d.ap(), fin_sb[:])

    nc.compile()
    return nc


# ----------------------------------------------------------------------------
# host side
# ----------------------------------------------------------------------------

_cache = {}


def _get_nc(use_mask, num_layers=NL):
    key = (use_mask, num_layers)
    if key not in _cache:
        _cache[key] = build_nc(use_mask, num_layers)
    return _cache[key]


def _bf(a):
    return np.ascontiguousarray(a).astype(ml_dtypes.bfloat16)


def prep_inputs(inputs, num_layers=NL):
    """Host-side prep: fold LN gains into the following matmuls, pre-arrange
    weights into contiguous DMA blocks, shard tokens across cores."""
    f = {k: np.asarray(v) for k, v in inputs.items()}
    src = f["src"].astype(np.float32)            # [B,S,IN]
    mask = np.asarray(f["mask"])
    use_mask = not bool((mask == 1).all())

    ln1_g, ln2_g, lnf_g = f["ln1_g"], f["ln2_g"], f["lnf_g"]

    # setup_inputs always uses zero biases / LN b; the device program carries
    # no bias adds, so require that here (fail loudly otherwise).
    for name in ("ln1_b", "ln2_b", "lnf_b", "bfc1", "bfc2", "bfc3", "bo",
                 "b1", "b2", "bout1", "bout2"):
        if np.abs(f[name]).max() != 0.0:
            raise NotImplementedError(f"nonzero bias {name} not supported")

    nl = num_layers
    wq = (f["Wq"] * ln1_g[:, :, None])[:nl]      # [nl,D,D]
    wk = (f["Wk"] * ln1_g[:, :, None])[:nl]
    wv = (f["Wv"] * ln1_g[:, :, None])[:nl]
    wo = f["Wo"][:nl]
    w1 = (f["W1"] * ln2_g[:, :, None])[:nl]      # [nl,D,DF]
    w2 = f["W2"][:nl]                            # [nl,DF,D]
    wout1 = f["Wout1"] * lnf_g[:, None]          # [D,DR]
    wout2 = f["Wout2"]                           # [DR,1]

    def pcf(w):  # [L,IN_,OF] -> [L,128,IN_/128,OF]
        L, i, o = w.shape
        return w.reshape(L, i // 128, 128, o).transpose(0, 2, 1, 3)

    wq_h, wk_h, wv_h = (
        _bf(pcf(w)).reshape(num_layers * 128, DC, D) for w in (wq, wk, wv))
    # wo 65-row head chunks (row 0 zero), of-blocks: [L, 4, 65, 16ci, 256]
    wo_r = wo.reshape(num_layers, 16, 64, 4, 256)
    wo65 = np.zeros((num_layers, 16, 65, 4, 256), dtype=np.float32)
    wo65[:, :, 1:, :, :] = wo_r
    wo_h = _bf(wo65.transpose(0, 3, 2, 1, 4).reshape(num_layers * 4 * 65, 16, 256))
    # w1 blocks [L, blk8, 128, 8ci, 512of]
    w1_h = _bf(w1.reshape(num_layers, DC, 128, 8, 512).transpose(0, 3, 2, 1, 4).reshape(num_layers * 8 * 128, DC, 512))
    # w2 blocks [L, co8, 128, 32ci, 128of]
    w2_h = _bf(w2.reshape(num_layers, DFC, 128, DC, 128).transpose(0, 3, 2, 1, 4).reshape(num_layers * 8 * 128, DFC, 128))
    wfc1_h = _bf(f["Wfc1"])                      # [64, 3072]
    # wfc2 blocks [12, 128, 24ci, 256of]
    wfc2_h = _bf(f["Wfc2"].reshape(24, 128, 24, 128).transpose(2, 1, 0, 3)
                 .reshape(24 * 128, 24, 128))
    wfc3_h = _bf(f["Wfc3"].reshape(24, 128, 8, 128).transpose(2, 1, 0, 3)
                 .reshape(8 * 128, 24, 128))
    wout1_h = _bf(wout1.reshape(DC, 128, DR).transpose(1, 0, 2))  # [128,8,256]
    wout2_h = _bf(wout2.reshape(2, 128, 1).transpose(1, 0, 2))    # [128,2,1]

    pe = _sinusoidal_pe(S, D)                    # [S,D]

    in_maps = []
    for core in range(NCORES):
        b = core // GRP
        t0 = (core % GRP) * T
        srcT = _bf(src[b, t0:t0 + T, :].T)       # [64, T]
        peT = np.ascontiguousarray(
            pe[t0:t0 + T, :].T).astype(np.float32)
        m = {
            "srcT": srcT, "peT": peT,
            "wfc1": wfc1_h, "wfc2": wfc2_h, "wfc3": wfc3_h,
            "wq": wq_h, "wk": wk_h, "wv": wv_h, "wo": wo_h,
            "w1": w1_h, "w2": w2_h,
            "wout1": wout1_h, "wout2": wout2_h,
        }
        if use_mask:
            mb = np.where(mask[b, t0:t0 + T, :] == 0, -8e9, 0.0).astype(np.float32)
            m["maskb"] = np.ascontiguousarray(mb.T)
        in_maps.append(m)
    return in_maps, use_mask


def kernel(**inputs):
    in_maps, use_mask = prep_inputs(inputs)
    nc = _get_nc(use_mask)
    res = bass_utils.run_bass_kernel_spmd(
        nc, in_maps, core_ids=list(range(NCORES)))
    out = np.concatenate(
        [res.results[i]["out"].reshape(-1) for i in range(NCORES)])
    return out.reshape(B, S, 1).astype(np.float32)
